# revision 37
# baseline (speedup 1.0000x reference)
"""Deformable multi-head sparse attention (DMSA) Bass kernel for Trainium2, v2.

Contract: kernel(**inputs) takes the FULL unsharded inputs (as produced by
setup_inputs()) and returns the FULL output (B, 384, 56, 56) float32.
Internally shards batch B=8 across 8 NeuronCores (pure data parallel,
no collectives), one batch element per core.

v2 redesign vs v1:
- bf16 operands for all big matmuls (q/dwconv/S/O/proj), exploiting the
  loose 2e-2 tolerance.
- Exp batched into 2 ACT instructions per (head, kv-chunk) over multi-bank
  PSUM APs (free size 1792/1344) instead of 7x448 -> ~25% less ACT busy.
- attention*V restructured: E (exp scores) becomes the matmul STATIONARY
  operand, V the moving one -> 65 rows/matmul instead of 448; softmax
  denominator rides along as a 65th V column and lands per-partition, so
  normalization is a cheap reciprocal + stride-0-broadcast multiply.
- padless depthwise conv: edge-clipped accumulating matmuls directly on q.
- gather bilinear weight rows broadcast via gpsimd partition_broadcast
  instead of PE matmul + ACT copies.
- engine rebalance: ACT owns exp; DVE owns element-wise; Pool owns gathers,
  broadcasts and part of the copies.

Self-contained: hardcodes all shapes; does not read any sibling files.
"""
import sys

for _p in ("/opt/trn_rl_repo", "/opt/pypackages"):
    if _p not in sys.path:
        sys.path.insert(0, _p)

import numpy as np

import concourse.bass as bass
import concourse.mybir as mybir
import concourse.tile as tile
from concourse import bacc
from concourse import bass_utils

F32 = mybir.dt.float32
F32R = mybir.dt.float32r
BF16 = mybir.dt.bfloat16
I16 = mybir.dt.int16
I32 = mybir.dt.int32
AF = mybir.ActivationFunctionType
OP = mybir.AluOpType

# problem constants
B = 8
DIM = 384
DIM_HEAD = 64
NUM_HEAD = 6
G = 3            # deformable groups
NGD = 128        # channels per group
H = 56
W = 56
HW = H * W       # 3136
HO = 28
WO = 28
L = HO * WO      # 784
SCALE = DIM_HEAD ** -0.5
BN_EPS = 1e-6
A = (W - 1) / WO   # 55/28, same for y since H==W and HO==WO

QC = 448           # q chunk for 1-bank psum slices
NQC = HW // QC     # 7
LC = 112           # kv-position chunk (partition dim of S^T)
NLC = L // LC      # 7
OCH = 128          # q-position chunk for the O phase (partition dim)
NOCH = 25          # ceil(3136/128); last chunk has 64 rows
ORND = [(0, 7), (7, 14), (14, 21), (21, 25)]  # O-phase chunk rounds (bank reuse)


def build_nc(dbg=()):
    """Build the per-core Bass program (SPMD: same NEFF on all 8 cores).

    dbg: tuple of intermediate names to expose as extra DRAM outputs.
    """
    nc = bacc.Bacc("TRN2", target_bir_lowering=False, debug=False, num_devices=B)

    din = {}
    def dt_in(name, shape, dtype=F32):
        din[name] = nc.dram_tensor(name, shape, dtype, kind="ExternalInput").ap()
        return din[name]

    dt_in("x", [DIM, HW], F32R)
    dt_in("qw_t", [DIM, DIM], F32R)
    dt_in("kwk_t", [DIM, DIM], BF16)
    dt_in("kwv_t", [DIM, DIM], BF16)
    dt_in("projw_t", [DIM, DIM], BF16)
    dt_in("pw_t", [NGD, 3])
    dt_in("projb_rs", [NGD, 3])
    dt_in("diag", [NGD, 25 * 128], BF16)
    dt_in("bn_s", [NGD, 1])
    dt_in("bn_t", [NGD, 1])
    dt_in("ident_b", [128, 128], BF16)
    dt_in("ytA", [LC, 21])
    dt_in("xtA", [LC, 21])
    dt_in("sel", [LC, 7 * 128], F32R)

    out_d = nc.dram_tensor("out", [DIM, HW], F32, kind="ExternalOutput").ap()

    with tile.TileContext(nc) as tc:
        _body(nc, tc, din, out_d, dbg)

    nc.compile()
    return nc


def _w3(tile_, kc, lo, hi):
    """Slice [128, (3,384)] weight tile to [128, hi-lo] for k-chunk kc."""
    return tile_[:].rearrange("p (c n) -> p c n", c=3)[:, kc, lo:hi]


def _body(nc, tc, din, out_d, dbg=()):
    import contextlib

    def dbg_out(name, tiles):
        """Expose tile(s) as an extra DRAM output (list stacked on dim 0)."""
        if name not in dbg:
            return
        if not isinstance(tiles, (list, tuple)):
            tiles = [tiles]
        sh = list(tiles[0].shape)
        dt = tiles[0].dtype
        d = nc.dram_tensor("dbg_" + name, [len(tiles)] + sh, dt,
                           kind="ExternalOutput").ap()
        for i, t in enumerate(tiles):
            nc.sync.dma_start(d[i], t[:])

    ctx = contextlib.ExitStack()
    with ctx:
        # persistent pools
        wpool = ctx.enter_context(tc.tile_pool(name="wpool", bufs=1))
        qpool = ctx.enter_context(tc.tile_pool(name="qpool", bufs=1))
        # OP psum bank must outlive the S psum pool (LIFO) -> open first
        oppsum = ctx.enter_context(tc.tile_pool(name="oppsum", bufs=1, space="PSUM"))
        dram = ctx.enter_context(tc.tile_pool(name="dram", bufs=1, space="DRAM"))

        # ---------------- phase A: input DMAs + conversions ----------------
        def load_small(key, shape, dtype=F32):
            t = wpool.tile(shape, dtype, name=key + "_sb")
            nc.sync.dma_start(t[:], din[key][:])
            return t

        # gather-phase pool: outlives the preamble psum pool (LIFO)
        xctx = contextlib.ExitStack()
        xpool = xctx.enter_context(tc.tile_pool(name="xpool", bufs=1))

        # weights: one DMA each into [128, (3,384)]; dtypes pre-converted on host
        def load_w3(key, dtype):
            t = wpool.tile([128, 3 * DIM], dtype, name=key + "_w")
            nc.sync.dma_start(
                t[:].rearrange("p (c n) -> p c n", c=3),
                din[key].rearrange("(c p) n -> p c n", c=3),
            )
            return t

        # DMA order matters: the SP queue + shared HWDGE serialize at ~625ns
        # per DMA, so emit what the first matmuls need first.
        qw_r = load_w3("qw_t", F32R)

        # x loads straight into f32r tiles (f32r == fp32 bits; PE rounds on use)
        x_sb = []
        for g in range(G):
            xs_ = xpool.tile([128, HW], F32R, name=f"x_sb{g}")
            nc.sync.dma_start(xs_[:], din["x"][128 * g:128 * (g + 1), :])
            x_sb.append(xs_)

        diag = load_small("diag", [NGD, 25 * 128], BF16)
        bns_sb = load_small("bn_s", [NGD, 1])
        bnt_sb = load_small("bn_t", [NGD, 1])
        pw_sb = load_small("pw_t", [NGD, 3])
        ytA_sb = load_small("ytA", [LC, 21])
        xtA_sb = load_small("xtA", [LC, 21])
        sel_sb = load_small("sel", [LC, 7 * 128], F32R)
        kwk_b = load_w3("kwk_t", BF16)
        kwv_b = load_w3("kwv_t", BF16)
        pjw_b = load_w3("projw_t", BF16)
        pjb_sb = load_small("projb_rs", [NGD, 3])
        ident_b = load_small("ident_b", [128, 128], BF16)

        # ---------------- phase B: q = q_w @ x (bf16 out) ----------------
        q_sb = [qpool.tile([128, HW], BF16, name=f"q_sb{m}") for m in range(3)]
        xs_sb = [qpool.tile([128, L], BF16, name=f"xs_sb{g}") for g in range(G)]
        ones128 = wpool.tile([1, 128], F32R, name="ones128")
        nc.vector.memset(ones128[:].bitcast(F32), 1.0)
        wgt_dr = dram.tile([G * 4 * NLC * LC], F32R)
        wgt_v = wgt_dr.rearrange("(g r c p) -> g p r c", g=G, r=4, c=NLC)
        wrow_v = wgt_dr.rearrange("(g o n) -> g o n", g=G, o=1)
        with tc.tile_pool(name="prepsum1", bufs=1, space="PSUM") as pre1:
            for m in range(3):
                for n in range(NQC):
                    pq = pre1.tile([128, QC], F32, tag="pq", bufs=2, name="pq")
                    for kc in range(3):
                        nc.tensor.matmul(
                            pq[:],
                            _w3(qw_r, kc, 128 * m, 128 * (m + 1)),
                            x_sb[kc][:, QC * n:QC * (n + 1)],
                            start=(kc == 0), stop=(kc == 2),
                        )
                    nc.vector.tensor_copy(q_sb[m][:, QC * n:QC * (n + 1)], pq[:])

                # ---- dwconv for group m right after q_sb[m] is complete ----
                g = m
                qv = q_sb[g][:].rearrange("p (h w) -> p h w", w=W)
                gelu = qpool.tile([128, L], F32, tag="gelu", bufs=3, name="gelu")
                for nn in range(2):
                    pdw = pre1.tile([128, 14 * 28], F32, tag="pdw", bufs=2, name="pdw")
                    pdv = pdw[:].rearrange("p (h w) -> p h w", w=28)
                    taps = [12] + [t for t in range(25) if t != 12]
                    for ti, t in enumerate(taps):
                        ty, tx = t // 5, t % 5
                        ya = max(1 if ty <= 1 else 0, 14 * nn)
                        yb = min(27 if ty == 4 else 28, 14 * nn + 14)
                        xa = 1 if tx <= 1 else 0
                        xb = 27 if tx == 4 else 28
                        rhs = qv[:, 2 * ya + ty - 2: 2 * (yb - 1) + ty - 1: 2,
                                 2 * xa + tx - 2: 2 * (xb - 1) + tx - 1: 2]
                        nc.tensor.matmul(
                            pdv[:, ya - 14 * nn: yb - 14 * nn, xa:xb],
                            diag[:, 128 * t:128 * (t + 1)],
                            rhs,
                            start=(ti == 0), stop=(ti == 24),
                        )
                    nc.scalar.activation(gelu[:, 392 * nn:392 * (nn + 1)], pdw[:],
                                         AF.Gelu, bias=bnt_sb[:, 0:1],
                                         scale=bns_sb[:, 0:1])

                # ---- om = pw @ gelu for this group ----
                pom = pre1.tile([LC, 21], F32, tag="pom", bufs=1, name="pom")
                for c in range(NLC):
                    nc.tensor.matmul(
                        pom[:, 3 * c:3 * (c + 1)],
                        gelu[:, LC * c:LC * (c + 1)],
                        pw_sb[:, 0:3],
                        start=True, stop=True,
                    )
                om_g = qpool.tile([LC, 21], F32, tag="om_g", bufs=2, name="om_g")
                nc.vector.tensor_copy(om_g[:], pom[:])

                # ---- per-group position math (overlaps next group's PE) ----
                omv = om_g[:].rearrange("p (c ch) -> p c ch", ch=3)
                om0, om1, om2 = omv[:, :, 0], omv[:, :, 1], omv[:, :, 2]

                def dvt(tag, dtype=F32):
                    return qpool.tile([LC, NLC], dtype, tag=tag, bufs=2,
                                      name=tag)

                ty_t = dvt("ty_t"); tx_t = dvt("tx_t")
                sg_t = dvt("sg_t"); mod_t = dvt("mod_t")
                nc.scalar.activation(ty_t[:], om0, AF.Tanh)
                nc.scalar.activation(tx_t[:], om1, AF.Tanh)
                nc.scalar.activation(sg_t[:], om2, AF.Sigmoid)
                nc.scalar.activation(mod_t[:], sg_t[:], AF.Sigmoid)

                gy = dvt("gy"); gx = dvt("gx")
                nc.vector.scalar_tensor_tensor(
                    gy[:], ty_t[:], float(A), ytA_sb[:, NLC * g:NLC * (g + 1)],
                    op0=OP.mult, op1=OP.add)
                nc.vector.scalar_tensor_tensor(
                    gx[:], tx_t[:], float(A), xtA_sb[:, NLC * g:NLC * (g + 1)],
                    op0=OP.mult, op1=OP.add)

                # floor: int copy rounds to nearest -> subtract (rounded > val)
                y0i = dvt("y0i", I32); x0i = dvt("x0i", I32)
                y0f = dvt("y0f"); x0f = dvt("x0f")
                rfx = dvt("rfx")
                nc.vector.tensor_copy(y0i[:], gy[:])
                nc.vector.tensor_copy(y0f[:], y0i[:])
                nc.vector.tensor_tensor(rfx[:], y0f[:], gy[:], op=OP.is_gt)
                nc.vector.tensor_tensor(y0f[:], y0f[:], rfx[:], op=OP.subtract)
                nc.vector.tensor_copy(x0i[:], gx[:])
                nc.vector.tensor_copy(x0f[:], x0i[:])
                nc.vector.tensor_tensor(rfx[:], x0f[:], gx[:], op=OP.is_gt)
                nc.vector.tensor_tensor(x0f[:], x0f[:], rfx[:], op=OP.subtract)
                fy = dvt("fy"); fx = dvt("fx")
                nc.vector.tensor_tensor(fy[:], gy[:], y0f[:], op=OP.subtract)
                nc.vector.tensor_tensor(fx[:], gx[:], x0f[:], op=OP.subtract)

                my0 = dvt("my0"); my1 = dvt("my1")
                mx0 = dvt("mx0"); mx1 = dvt("mx1")
                nc.vector.tensor_scalar(my0[:], gy[:], 2.0, None, OP.is_ge)
                nc.vector.tensor_scalar(my1[:], gy[:], 57.0, None, OP.is_lt)
                nc.vector.tensor_scalar(mx0[:], gx[:], 2.0, None, OP.is_ge)
                nc.vector.tensor_scalar(mx1[:], gx[:], 57.0, None, OP.is_lt)

                wy0 = dvt("wy0"); wy1 = dvt("wy1")
                wx0 = dvt("wx0"); wx1 = dvt("wx1")
                omf = dvt("omf")
                nc.vector.tensor_scalar(omf[:], fy[:], -1.0, 1.0, OP.mult, OP.add)
                nc.vector.tensor_tensor(wy0[:], omf[:], my0[:], op=OP.mult)
                nc.vector.tensor_tensor(wy0[:], wy0[:], mod_t[:], op=OP.mult)
                nc.vector.tensor_tensor(wy1[:], fy[:], my1[:], op=OP.mult)
                nc.vector.tensor_tensor(wy1[:], wy1[:], mod_t[:], op=OP.mult)
                nc.vector.tensor_scalar(omf[:], fx[:], -1.0, 1.0, OP.mult, OP.add)
                nc.vector.tensor_tensor(wx0[:], omf[:], mx0[:], op=OP.mult)
                nc.vector.tensor_tensor(wx1[:], fx[:], mx1[:], op=OP.mult)

                Wt_g = qpool.tile([LC, 4 * NLC], F32R, tag="Wt_g", bufs=2,
                                  name="Wt_g")
                Wv = Wt_g[:].rearrange("p (r c) -> p r c", r=4)
                nc.vector.tensor_tensor(Wv[:, 0, :], wy0[:], wx0[:], op=OP.mult)
                nc.vector.tensor_tensor(Wv[:, 1, :], wy0[:], wx1[:], op=OP.mult)
                nc.vector.tensor_tensor(Wv[:, 2, :], wy1[:], wx0[:], op=OP.mult)
                nc.vector.tensor_tensor(Wv[:, 3, :], wy1[:], wx1[:], op=OP.mult)

                yc0 = dvt("yc0"); yc1 = dvt("yc1")
                xc0 = dvt("xc0"); xc1 = dvt("xc1")
                nc.vector.tensor_scalar(yc0[:], y0f[:], -2.0, 0.0, OP.add, OP.max)
                nc.vector.tensor_scalar(yc0[:], yc0[:], 55.0, 56.0, OP.min, OP.mult)
                nc.vector.tensor_scalar(yc1[:], y0f[:], -1.0, 0.0, OP.add, OP.max)
                nc.vector.tensor_scalar(yc1[:], yc1[:], 55.0, 56.0, OP.min, OP.mult)
                nc.vector.tensor_scalar(xc0[:], x0f[:], -2.0, 0.0, OP.add, OP.max)
                nc.vector.tensor_scalar(xc0[:], xc0[:], 55.0, None, OP.min)
                nc.vector.tensor_scalar(xc1[:], x0f[:], -1.0, 0.0, OP.add, OP.max)
                nc.vector.tensor_scalar(xc1[:], xc1[:], 55.0, None, OP.min)

                If_g = qpool.tile([LC, 4 * NLC], F32R, tag="If_g", bufs=2,
                                  name="If_g")
                Ifv = If_g[:].rearrange("p (r c) -> p r c", r=4)
                nc.vector.tensor_tensor(Ifv[:, 0, :], yc0[:], xc0[:], op=OP.add)
                nc.vector.tensor_tensor(Ifv[:, 1, :], yc0[:], xc1[:], op=OP.add)
                nc.vector.tensor_tensor(Ifv[:, 2, :], yc1[:], xc0[:], op=OP.add)
                nc.vector.tensor_tensor(Ifv[:, 3, :], yc1[:], xc1[:], op=OP.add)

                # ---- idx wrap via PE selector matmuls (no DRAM roundtrip) ----
                pidx = pre1.tile([128, 196], F32, tag="pidx", bufs=1, name="pidx")
                pidxv = pidx[:].rearrange("p (ph rc) -> p ph rc", ph=7)
                for ph in range(7):
                    nc.tensor.matmul(
                        pidxv[:, ph, :],
                        sel_sb[:, 128 * ph:128 * (ph + 1)],
                        If_g[:],
                        start=True, stop=True,
                    )
                idxw = xpool.tile([128, 196], I16, tag="idxw", bufs=3, name="idxw")
                nc.vector.tensor_copy(
                    idxw[:].rearrange("p (rc ph) -> p ph rc", ph=7), pidxv[:])

                # ---- bilinear weight rows: DRAM flatten + PE broadcast ----
                nc.sync.dma_start(wgt_v[g], Wt_g[:].rearrange(
                    "p (r c) -> p r c", r=4))
                wrow1 = xpool.tile([1, 4 * L], F32R, tag="wrow1", bufs=1,
                                   name="wrow1")
                nc.sync.dma_start(wrow1[:], wrow_v[g])
                wbc = []
                for r in range(4):
                    wb = xpool.tile([128, L], F32, tag="wbc", bufs=6, name="wbc")
                    for n2 in range(2):
                        pwb = pre1.tile([128, 392], F32, tag="pwb", bufs=1,
                                        name="pwb")
                        nc.tensor.matmul(
                            pwb[:], ones128[:],
                            wrow1[0:1, L * r + 392 * n2:L * r + 392 * (n2 + 1)],
                            start=True, stop=True,
                        )
                        nc.scalar.activation(wb[:, 392 * n2:392 * (n2 + 1)],
                                             pwb[:], AF.Copy)
                    wbc.append(wb)

                # ---- gather + in-place bilinear combine ----
                gat = xpool.tile([128, 4 * L], F32, tag="gat", bufs=2, name="gat")
                nc.gpsimd.ap_gather(
                    gat[:], x_sb[g][:].bitcast(F32), idxw[:],
                    channels=128, num_elems=HW, d=1, num_idxs=4 * L,
                )
                gv = gat[:].rearrange("p (r n) -> p r n", r=4)
                for r in range(4):
                    nc.vector.tensor_tensor(gv[:, r, :], gv[:, r, :], wbc[r][:],
                                            op=OP.mult)
                nc.vector.tensor_tensor(gv[:, 0, :], gv[:, 0, :], gv[:, 1, :],
                                        op=OP.add)
                nc.vector.tensor_tensor(gv[:, 2, :], gv[:, 2, :], gv[:, 3, :],
                                        op=OP.add)
                nc.vector.tensor_tensor(xs_sb[g][:], gv[:, 0, :], gv[:, 2, :],
                                        op=OP.add)

            dbg_out("q", q_sb)
            dbg_out("xs", xs_sb)

        xctx.close()   # release x / gather staging space

        # ---------------- phase H: k and v^T (bf16) ----------------
        k_sb = [qpool.tile([128, L], BF16, name=f"k_sb{m}") for m in range(3)]
        vTe = [qpool.tile([LC, 6 * 65], BF16, name=f"vTe{lc}") for lc in range(NLC)]
        with tc.tile_pool(name="prepsum2", bufs=1, space="PSUM") as pre2:
            for m in range(3):
                for nn in range(2):
                    pk = pre2.tile([128, 392], F32, tag="pk", bufs=2, name="pk")
                    for kc in range(3):
                        nc.tensor.matmul(
                            pk[:],
                            _w3(kwk_b, kc, 128 * m, 128 * (m + 1)),
                            xs_sb[kc][:, 392 * nn:392 * (nn + 1)],
                            start=(kc == 0), stop=(kc == 2),
                        )
                    nc.vector.tensor_copy(k_sb[m][:, 392 * nn:392 * (nn + 1)], pk[:])

            for lc in range(NLC):
                nc.vector.memset(vTe[lc][:].bitcast(mybir.dt.uint16), 0x3F80)
                pv = pre2.tile([LC, DIM], F32, tag="pv", bufs=2, name="pv")
                for kc in range(3):
                    nc.tensor.matmul(
                        pv[:],
                        xs_sb[kc][:, LC * lc:LC * (lc + 1)],
                        _w3(kwv_b, kc, 0, DIM),
                        start=(kc == 0), stop=(kc == 2),
                    )
                dst = vTe[lc][:].rearrange("p (h d) -> p h d", h=6)[:, :, 0:64]
                nc.vector.tensor_copy(dst, pv[:].rearrange("p (h d) -> p h d", h=6))

        # memset 0x3F80 writes bf16 1.0 everywhere; cols 0:64 overwritten above

        dbg_out("k", k_sb)
        dbg_out("v", vTe)

        # ---------------- phase I: attention ----------------
        ot_all = qpool.tile([128, NOCH * DIM], BF16, name="ot_all")
        otv = ot_all[:].rearrange("p (c n) -> p c n", c=NOCH)
        OPp = oppsum.tile([128, 7 * 65], F32, name="OPp")
        OPv = OPp[:].rearrange("p (c e) -> p c e", c=7)

        def o_round(h, rnd, Eh):
            c0, c1 = ORND[rnd]
            nch = c1 - c0
            for ci in range(nch):
                c = c0 + ci
                qn = 128 if c < NOCH - 1 else 64
                for lc in range(NLC):
                    nc.tensor.matmul(
                        OPv[0:qn, ci, :],
                        Eh[lc][:, OCH * c:OCH * c + qn],
                        vTe[lc][:].rearrange("p (hh d) -> p hh d", hh=6)[:, h, :],
                        start=(lc == 0), stop=(lc == NLC - 1),
                    )
            # normalize: rec = 1/denom  (denom = col 64), out = O * rec
            nfull = nch if c1 < NOCH else nch - 1
            rec = qpool.tile([128, 7], F32, tag="rec", bufs=3, name="rec")
            if nfull > 0:
                nc.vector.reciprocal(rec[:, 0:nfull], OPv[:, 0:nfull, 64:65])
                nc.vector.tensor_tensor(
                    otv[:, c0:c0 + nfull, 64 * h:64 * (h + 1)],
                    OPv[:, 0:nfull, 0:64],
                    rec[:, 0:nfull].rearrange("p (c o) -> p c o", o=1)
                       .to_broadcast([128, nfull, 64]),
                    op=OP.mult)
            if c1 == NOCH:  # trailing 64-row chunk
                ci = nch - 1
                nc.vector.reciprocal(rec[0:64, ci:ci + 1], OPv[0:64, ci, 64:65])
                nc.vector.tensor_tensor(
                    otv[0:64, NOCH - 1, 64 * h:64 * (h + 1)],
                    OPv[0:64, ci, 0:64],
                    rec[0:64, ci:ci + 1].rearrange("p (c o) -> p c o", o=1)
                       .to_broadcast([64, 1, 64]),
                    op=OP.mult)

        Eprev = None
        with tc.tile_pool(name="spsum", bufs=1, space="PSUM") as spsum, \
             tc.tile_pool(name="epool", bufs=1) as epool:
            SA = spsum.tile([LC, 4 * 512], F32, name="SA")
            SB = spsum.tile([LC, 3 * 512], F32, name="SB")
            SAv = SA[:].rearrange("p (i n) -> p i n", i=4)
            SBv = SB[:].rearrange("p (i n) -> p i n", i=3)
            for h in range(NUM_HEAD):
                m2, hh = h // 2, h % 2
                kp = k_sb[m2][64 * hh:64 * hh + 64, :]
                qp = q_sb[m2][64 * hh:64 * hh + 64, :]
                Eh = []
                for lc in range(NLC):
                    for qi in range(NQC):
                        dst = SAv[:, qi, 0:QC] if qi < 4 else SBv[:, qi - 4, 0:QC]
                        nc.tensor.matmul(
                            dst,
                            kp[:, LC * lc:LC * (lc + 1)],
                            qp[:, QC * qi:QC * (qi + 1)],
                            start=True, stop=True,
                        )
                    E = epool.tile([LC, HW], BF16, tag=f"E{lc}", bufs=2, name="E")
                    Ev = E[:].rearrange("p (i n) -> p i n", i=NQC)
                    nc.scalar.activation(Ev[:, 0:4, :], SAv[:, :, 0:QC], AF.Exp)
                    nc.scalar.activation(Ev[:, 4:7, :], SBv[:, :, 0:QC], AF.Exp)
                    Eh.append(E)
                    if h == 0 and lc == NLC - 1:
                        dbg_out("E_h0", Eh)
                    if Eprev is not None and lc < 4:
                        o_round(h - 1, lc, Eprev)
                Eprev = Eh
            for rnd in range(4):
                o_round(NUM_HEAD - 1, rnd, Eprev)

        dbg_out("ot", ot_all)

        # ---------------- phase J: transpose O^T -> O ----------------
        O_sb = [qpool.tile([128, HW], BF16, name=f"O_sb{j}") for j in range(3)]
        with tc.tile_pool(name="tailpsum", bufs=1, space="PSUM") as tpsum:
            for c in range(NOCH):
                pn = 128 if c < NOCH - 1 else 64
                for j in range(3):
                    tp = tpsum.tile([128, 128], BF16, tag="tp", bufs=4, name="tp")
                    nc.tensor.transpose(tp[0:128, 0:pn],
                                        otv[0:pn, c, 128 * j:128 * (j + 1)],
                                        ident_b[0:pn, 0:pn])
                    dst = O_sb[j][:, OCH * c:OCH * c + pn]
                    nc.vector.tensor_copy(dst, tp[0:128, 0:pn])

            dbg_out("O", O_sb)

            # ---------------- phase K: proj + bias + out DMA ----------------
            with tc.tile_pool(name="outpool", bufs=1) as outpool:
                out_sb = [outpool.tile([128, HW], F32, name=f"out_sb{m}")
                          for m in range(3)]
                for m in range(3):
                    for n in range(NQC):
                        pp = tpsum.tile([128, QC], F32, tag="pp", bufs=2, name="pp")
                        for kc in range(3):
                            nc.tensor.matmul(
                                pp[:],
                                _w3(pjw_b, kc, 128 * m, 128 * (m + 1)),
                                O_sb[kc][:, QC * n:QC * (n + 1)],
                                start=(kc == 0), stop=(kc == 2),
                            )
                        nc.scalar.activation(out_sb[m][:, QC * n:QC * (n + 1)],
                                             pp[:], AF.Identity,
                                             bias=pjb_sb[:, m:m + 1])
                    nc.sync.dma_start(out_d[128 * m:128 * (m + 1), :],
                                      out_sb[m][:])


def host_prep(inputs):
    """Shared (per-core-identical) weight prep. Returns dict of np arrays."""
    f = np.float32
    q_w = np.asarray(inputs["q_w"], f)
    kv_w = np.asarray(inputs["kv_w"], f)
    proj_w = np.asarray(inputs["proj_w"], f)
    proj_b = np.asarray(inputs["proj_b"], f)
    dw_w = np.asarray(inputs["dw_w"], f)
    dw_b = np.asarray(inputs["dw_b"], f)
    bn_w = np.asarray(inputs["bn_w"], f)
    bn_b = np.asarray(inputs["bn_b"], f)
    bn_mean = np.asarray(inputs["bn_mean"], f)
    bn_var = np.asarray(inputs["bn_var"], f)
    pw_w = np.asarray(inputs["pw_w"], f)

    bn_s = (bn_w / np.sqrt(bn_var + BN_EPS)).astype(f)
    bn_t = ((dw_b - bn_mean) * bn_s + bn_b).astype(f)

    p = np.arange(LC)
    c = np.arange(NLC)
    ytab_col = (4 * c[None, :] + p[:, None] // 28 + 0.5 + 2.0 / A).astype(f)
    ytab = np.tile(ytab_col, (1, G))                       # [112, 21] (g, c)
    xtab_col = (p % 28 + 0.5 + 2.0 / A).astype(f)[:, None]
    xtab = np.tile(xtab_col, (1, G * NLC))

    import ml_dtypes
    bf = ml_dtypes.bfloat16
    dwf = dw_w.reshape(NGD, 25)
    diag = np.zeros((NGD, 25, 128), f)
    diag[np.arange(128), :, np.arange(128)] = dwf
    # idx-wrap selector: sel[k, ph*128 + j] = 1 iff k == 16*ph + (j % 16)
    sel = np.zeros((LC, 7, 128), f)
    k_i = np.arange(LC)
    for ph in range(7):
        for j in range(128):
            sel[16 * ph + (j % 16), ph, j] = 1.0
    sel = np.ascontiguousarray(sel.reshape(LC, 7 * 128))
    return {
        "qw_t": np.ascontiguousarray(q_w.T),
        "kwk_t": np.ascontiguousarray((kv_w[:DIM] * SCALE).T).astype(bf),
        "kwv_t": np.ascontiguousarray(kv_w[DIM:].T).astype(bf),
        "pw_t": np.ascontiguousarray(pw_w.T),
        "projw_t": np.ascontiguousarray(proj_w.T).astype(bf),
        "projb_rs": np.ascontiguousarray(proj_b.reshape(3, NGD).T),
        "diag": np.ascontiguousarray(diag.reshape(NGD, 25 * 128)).astype(bf),
        "bn_s": bn_s.reshape(NGD, 1),
        "bn_t": bn_t.reshape(NGD, 1),
        "ident_b": np.eye(128, dtype=f).astype(bf),
        "ytA": (ytab * A).astype(f),
        "xtA": (xtab * A).astype(f),
        "sel": sel,
    }


_NC_CACHE = {}


def _get_nc(dummy=True):
    if "nc" not in _NC_CACHE:
        _NC_CACHE["nc"] = build_nc()
    return _NC_CACHE["nc"]


def make_in_maps(inputs):
    shared = host_prep(inputs)
    x = np.asarray(inputs["x"], np.float32)
    in_maps = []
    for i in range(B):
        m = dict(shared)
        m["x"] = np.ascontiguousarray(x[i].reshape(DIM, HW))
        in_maps.append(m)
    return in_maps


def run_spmd(inputs, trace=False):
    """Run on the 8 NeuronCores; returns (out (8,384,56,56), BassKernelResults)."""
    nc = _get_nc()
    in_maps = make_in_maps(inputs)
    res = bass_utils.run_bass_kernel_spmd(
        nc, in_maps, core_ids=list(range(B)), trace=trace,
    )
    out = np.stack([r["out"].reshape(DIM, H, W) for r in res.results], axis=0)
    return out, res


def kernel(**inputs) -> np.ndarray:
    out, _ = run_spmd(inputs, trace=False)
    return out


# revision 42
# speedup vs baseline: 1.0491x; 1.0491x over previous
"""Deformable multi-head sparse attention (DMSA) Bass kernel for Trainium2, v2.

Contract: kernel(**inputs) takes the FULL unsharded inputs (as produced by
setup_inputs()) and returns the FULL output (B, 384, 56, 56) float32.
Internally shards batch B=8 across 8 NeuronCores (pure data parallel,
no collectives), one batch element per core.

v2 redesign vs v1:
- bf16 operands for all big matmuls (q/dwconv/S/O/proj), exploiting the
  loose 2e-2 tolerance.
- Exp batched into 2 ACT instructions per (head, kv-chunk) over multi-bank
  PSUM APs (free size 1792/1344) instead of 7x448 -> ~25% less ACT busy.
- attention*V restructured: E (exp scores) becomes the matmul STATIONARY
  operand, V the moving one -> 65 rows/matmul instead of 448; softmax
  denominator rides along as a 65th V column and lands per-partition, so
  normalization is a cheap reciprocal + stride-0-broadcast multiply.
- padless depthwise conv: edge-clipped accumulating matmuls directly on q.
- gather bilinear weight rows broadcast via gpsimd partition_broadcast
  instead of PE matmul + ACT copies.
- engine rebalance: ACT owns exp; DVE owns element-wise; Pool owns gathers,
  broadcasts and part of the copies.

Self-contained: hardcodes all shapes; does not read any sibling files.
"""
import sys

for _p in ("/opt/trn_rl_repo", "/opt/pypackages"):
    if _p not in sys.path:
        sys.path.insert(0, _p)

import ml_dtypes
import numpy as np

import concourse.bass as bass
import concourse.mybir as mybir
import concourse.tile as tile
from concourse import bacc
from concourse import bass_utils

F32 = mybir.dt.float32
F32R = mybir.dt.float32r
BF16 = mybir.dt.bfloat16
I16 = mybir.dt.int16
I32 = mybir.dt.int32
AF = mybir.ActivationFunctionType
OP = mybir.AluOpType

# problem constants
B = 8
DIM = 384
DIM_HEAD = 64
NUM_HEAD = 6
G = 3            # deformable groups
NGD = 128        # channels per group
H = 56
W = 56
HW = H * W       # 3136
HO = 28
WO = 28
L = HO * WO      # 784
SCALE = DIM_HEAD ** -0.5
BN_EPS = 1e-6
A = (W - 1) / WO   # 55/28, same for y since H==W and HO==WO

QC = 448           # q chunk for 1-bank psum slices
NQC = HW // QC     # 7
LC = 112           # kv-position chunk (partition dim of S^T)
NLC = L // LC      # 7
OCH = 128          # q-position chunk for the O phase (partition dim)
NOCH = 25          # ceil(3136/128); last chunk has 64 rows
ORND = [(0, 7), (7, 14), (14, 21), (21, 25)]  # O-phase chunk rounds (bank reuse)


def build_nc(dbg=()):
    """Build the per-core Bass program (SPMD: same NEFF on all 8 cores).

    dbg: tuple of intermediate names to expose as extra DRAM outputs.
    """
    nc = bacc.Bacc("TRN2", target_bir_lowering=False, debug=False, num_devices=B)

    din = {}
    def dt_in(name, shape, dtype=F32):
        din[name] = nc.dram_tensor(name, shape, dtype, kind="ExternalInput").ap()
        return din[name]

    dt_in("x", [DIM, HW], F32R)
    dt_in("x_bf", [DIM, HW], BF16)
    dt_in("qw_t", [DIM, DIM], BF16)
    dt_in("kwk_t", [DIM, DIM], BF16)
    dt_in("kwv_t", [DIM, DIM], BF16)
    dt_in("projw_t", [DIM, DIM], BF16)
    dt_in("pw_t", [NGD, 3])
    dt_in("projb_rs", [NGD, 3])
    dt_in("diag", [NGD, 25 * 128], BF16)
    dt_in("bn_s", [NGD, 1])
    dt_in("bn_t", [NGD, 1])
    dt_in("ident_b", [128, 128], BF16)
    dt_in("ytA", [LC, 21])
    dt_in("xtA", [LC, 21])
    dt_in("sel", [LC, 7 * 128], F32R)

    out_d = nc.dram_tensor("out", [DIM, HW], F32, kind="ExternalOutput").ap()

    with tile.TileContext(nc) as tc:
        _body(nc, tc, din, out_d, dbg)

    nc.compile()
    return nc


def _w3(tile_, kc, lo, hi):
    """Slice [128, (3,384)] weight tile to [128, hi-lo] for k-chunk kc."""
    return tile_[:].rearrange("p (c n) -> p c n", c=3)[:, kc, lo:hi]


def _body(nc, tc, din, out_d, dbg=()):
    import contextlib

    def dbg_out(name, tiles):
        """Expose tile(s) as an extra DRAM output (list stacked on dim 0)."""
        if name not in dbg:
            return
        if not isinstance(tiles, (list, tuple)):
            tiles = [tiles]
        sh = list(tiles[0].shape)
        dt = tiles[0].dtype
        d = nc.dram_tensor("dbg_" + name, [len(tiles)] + sh, dt,
                           kind="ExternalOutput").ap()
        for i, t in enumerate(tiles):
            nc.sync.dma_start(d[i], t[:])

    ctx = contextlib.ExitStack()
    with ctx:
        # persistent pools
        wpool = ctx.enter_context(tc.tile_pool(name="wpool", bufs=1))
        qpool = ctx.enter_context(tc.tile_pool(name="qpool", bufs=1))
        # OP psum bank must outlive the S psum pool (LIFO) -> open first
        oppsum = ctx.enter_context(tc.tile_pool(name="oppsum", bufs=1, space="PSUM"))
        dram = ctx.enter_context(tc.tile_pool(name="dram", bufs=1, space="DRAM"))

        # ---------------- phase A: input DMAs + conversions ----------------
        def load_small(key, shape, dtype=F32):
            t = wpool.tile(shape, dtype, name=key + "_sb")
            nc.sync.dma_start(t[:], din[key][:])
            return t

        # gather-phase pool: outlives the preamble psum pool (LIFO)
        xctx = contextlib.ExitStack()
        xpool = xctx.enter_context(tc.tile_pool(name="xpool", bufs=1))

        # weights: one DMA each into [128, (3,384)]; dtypes pre-converted on host
        def load_w3(key, dtype):
            t = wpool.tile([128, 3 * DIM], dtype, name=key + "_w")
            nc.sync.dma_start(
                t[:].rearrange("p (c n) -> p c n", c=3),
                din[key].rearrange("(c p) n -> p c n", c=3),
            )
            return t

        # DMA order matters: the SP queue + shared HWDGE serialize at ~625ns
        # per DMA, so emit what the first matmuls need first.
        # bf16 x copy loads first (half the bytes) so q matmuls start early;
        # the fp32 x needed by the gather loads afterwards, hidden behind q/dw.
        xb_sb = []
        for g in range(G):
            xb_ = xpool.tile([128, HW], BF16, tag=f"x_bf{g}", bufs=1, name="xb")
            nc.sync.dma_start(xb_[:], din["x_bf"][128 * g:128 * (g + 1), :])
            xb_sb.append(xb_)
        qw_r = load_w3("qw_t", BF16)
        diag = load_small("diag", [NGD, 25 * 128], BF16)
        bns_sb = load_small("bn_s", [NGD, 1])
        bnt_sb = load_small("bn_t", [NGD, 1])
        pw_sb = load_small("pw_t", [NGD, 3])
        ytA_sb = load_small("ytA", [LC, 21])
        xtA_sb = load_small("xtA", [LC, 21])
        sel_sb = load_small("sel", [LC, 7 * 128], F32R)
        kwk_b = load_w3("kwk_t", BF16)
        kwv_b = load_w3("kwv_t", BF16)
        pjw_b = load_w3("projw_t", BF16)
        pjb_sb = load_small("projb_rs", [NGD, 3])
        ident_b = load_small("ident_b", [128, 128], BF16)

        # fp32 x (f32r bits) for the gather source; ready well before use
        x_sb = []
        for g in range(G):
            xs_ = xpool.tile([128, HW], F32R, name=f"x_sb{g}")
            nc.sync.dma_start(xs_[:], din["x"][128 * g:128 * (g + 1), :])
            x_sb.append(xs_)

        # ---------------- phase B: q = q_w @ x (bf16 out) ----------------
        q_sb = [qpool.tile([128, HW], BF16, name=f"q_sb{m}") for m in range(3)]
        xs_sb = [qpool.tile([128, L], BF16, name=f"xs_sb{g}") for g in range(G)]
        ones128 = wpool.tile([1, 128], F32R, name="ones128")
        nc.vector.memset(ones128[:].bitcast(F32), 1.0)
        wgt_dr = dram.tile([G * 4 * NLC * LC], F32R)
        wgt_v = wgt_dr.rearrange("(g r c p) -> g p r c", g=G, r=4, c=NLC)
        wrow_v = wgt_dr.rearrange("(g o n) -> g o n", g=G, o=1)
        with tc.tile_pool(name="prepsum1", bufs=1, space="PSUM") as pre1:
            for m in range(3):
                for n in range(NQC):
                    pq = pre1.tile([128, QC], F32, tag="pq", bufs=2, name="pq")
                    for kc in range(3):
                        nc.tensor.matmul(
                            pq[:],
                            _w3(qw_r, kc, 128 * m, 128 * (m + 1)),
                            xb_sb[kc][:, QC * n:QC * (n + 1)],
                            start=(kc == 0), stop=(kc == 2),
                        )
                    nc.vector.tensor_copy(q_sb[m][:, QC * n:QC * (n + 1)], pq[:])

                # ---- dwconv for group m right after q_sb[m] is complete ----
                g = m
                qv = q_sb[g][:].rearrange("p (h w) -> p h w", w=W)
                gelu = qpool.tile([128, L], F32, tag="gelu", bufs=3, name="gelu")
                for nn in range(2):
                    pdw = pre1.tile([128, 14 * 28], F32, tag="pdw", bufs=2, name="pdw")
                    pdv = pdw[:].rearrange("p (h w) -> p h w", w=28)
                    taps = [12] + [t for t in range(25) if t != 12]
                    for ti, t in enumerate(taps):
                        ty, tx = t // 5, t % 5
                        ya = max(1 if ty <= 1 else 0, 14 * nn)
                        yb = min(27 if ty == 4 else 28, 14 * nn + 14)
                        xa = 1 if tx <= 1 else 0
                        xb = 27 if tx == 4 else 28
                        rhs = qv[:, 2 * ya + ty - 2: 2 * (yb - 1) + ty - 1: 2,
                                 2 * xa + tx - 2: 2 * (xb - 1) + tx - 1: 2]
                        nc.tensor.matmul(
                            pdv[:, ya - 14 * nn: yb - 14 * nn, xa:xb],
                            diag[:, 128 * t:128 * (t + 1)],
                            rhs,
                            start=(ti == 0), stop=(ti == 24),
                        )
                    nc.scalar.activation(gelu[:, 392 * nn:392 * (nn + 1)], pdw[:],
                                         AF.Gelu, bias=bnt_sb[:, 0:1],
                                         scale=bns_sb[:, 0:1])

                # ---- om = pw @ gelu for this group ----
                pom = pre1.tile([LC, 21], F32, tag="pom", bufs=1, name="pom")
                for c in range(NLC):
                    nc.tensor.matmul(
                        pom[:, 3 * c:3 * (c + 1)],
                        gelu[:, LC * c:LC * (c + 1)],
                        pw_sb[:, 0:3],
                        start=True, stop=True,
                    )
                om_g = qpool.tile([LC, 21], F32, tag="om_g", bufs=2, name="om_g")
                nc.vector.tensor_copy(om_g[:], pom[:])

                # ---- per-group position math (overlaps next group's PE) ----
                omv = om_g[:].rearrange("p (c ch) -> p c ch", ch=3)
                om0, om1, om2 = omv[:, :, 0], omv[:, :, 1], omv[:, :, 2]

                def dvt(tag, dtype=F32):
                    return qpool.tile([LC, NLC], dtype, tag=tag, bufs=2,
                                      name=tag)

                ty_t = dvt("ty_t"); tx_t = dvt("tx_t")
                sg_t = dvt("sg_t"); mod_t = dvt("mod_t")
                nc.scalar.activation(ty_t[:], om0, AF.Tanh)
                nc.scalar.activation(tx_t[:], om1, AF.Tanh)
                nc.scalar.activation(sg_t[:], om2, AF.Sigmoid)
                nc.scalar.activation(mod_t[:], sg_t[:], AF.Sigmoid)

                gy = dvt("gy"); gx = dvt("gx")
                nc.vector.scalar_tensor_tensor(
                    gy[:], ty_t[:], float(A), ytA_sb[:, NLC * g:NLC * (g + 1)],
                    op0=OP.mult, op1=OP.add)
                nc.vector.scalar_tensor_tensor(
                    gx[:], tx_t[:], float(A), xtA_sb[:, NLC * g:NLC * (g + 1)],
                    op0=OP.mult, op1=OP.add)

                # floor: int copy rounds to nearest -> subtract (rounded > val)
                y0i = dvt("y0i", I32); x0i = dvt("x0i", I32)
                y0f = dvt("y0f"); x0f = dvt("x0f")
                rfx = dvt("rfx")
                nc.vector.tensor_copy(y0i[:], gy[:])
                nc.vector.tensor_copy(y0f[:], y0i[:])
                nc.vector.tensor_tensor(rfx[:], y0f[:], gy[:], op=OP.is_gt)
                nc.vector.tensor_tensor(y0f[:], y0f[:], rfx[:], op=OP.subtract)
                nc.vector.tensor_copy(x0i[:], gx[:])
                nc.vector.tensor_copy(x0f[:], x0i[:])
                nc.vector.tensor_tensor(rfx[:], x0f[:], gx[:], op=OP.is_gt)
                nc.vector.tensor_tensor(x0f[:], x0f[:], rfx[:], op=OP.subtract)
                fy = dvt("fy"); fx = dvt("fx")
                nc.vector.tensor_tensor(fy[:], gy[:], y0f[:], op=OP.subtract)
                nc.vector.tensor_tensor(fx[:], gx[:], x0f[:], op=OP.subtract)

                my0 = dvt("my0"); my1 = dvt("my1")
                mx0 = dvt("mx0"); mx1 = dvt("mx1")
                nc.vector.tensor_scalar(my0[:], gy[:], 2.0, None, OP.is_ge)
                nc.vector.tensor_scalar(my1[:], gy[:], 57.0, None, OP.is_lt)
                nc.vector.tensor_scalar(mx0[:], gx[:], 2.0, None, OP.is_ge)
                nc.vector.tensor_scalar(mx1[:], gx[:], 57.0, None, OP.is_lt)

                wy0 = dvt("wy0"); wy1 = dvt("wy1")
                wx0 = dvt("wx0"); wx1 = dvt("wx1")
                omf = dvt("omf")
                nc.vector.tensor_scalar(omf[:], fy[:], -1.0, 1.0, OP.mult, OP.add)
                nc.vector.tensor_tensor(wy0[:], omf[:], my0[:], op=OP.mult)
                nc.vector.tensor_tensor(wy0[:], wy0[:], mod_t[:], op=OP.mult)
                nc.vector.tensor_tensor(wy1[:], fy[:], my1[:], op=OP.mult)
                nc.vector.tensor_tensor(wy1[:], wy1[:], mod_t[:], op=OP.mult)
                nc.vector.tensor_scalar(omf[:], fx[:], -1.0, 1.0, OP.mult, OP.add)
                nc.vector.tensor_tensor(wx0[:], omf[:], mx0[:], op=OP.mult)
                nc.vector.tensor_tensor(wx1[:], fx[:], mx1[:], op=OP.mult)

                Wt_g = qpool.tile([LC, 4 * NLC], F32R, tag="Wt_g", bufs=2,
                                  name="Wt_g")
                Wv = Wt_g[:].rearrange("p (r c) -> p r c", r=4)
                nc.vector.tensor_tensor(Wv[:, 0, :], wy0[:], wx0[:], op=OP.mult)
                nc.vector.tensor_tensor(Wv[:, 1, :], wy0[:], wx1[:], op=OP.mult)
                nc.vector.tensor_tensor(Wv[:, 2, :], wy1[:], wx0[:], op=OP.mult)
                nc.vector.tensor_tensor(Wv[:, 3, :], wy1[:], wx1[:], op=OP.mult)

                yc0 = dvt("yc0"); yc1 = dvt("yc1")
                xc0 = dvt("xc0"); xc1 = dvt("xc1")
                nc.vector.tensor_scalar(yc0[:], y0f[:], -2.0, 0.0, OP.add, OP.max)
                nc.vector.tensor_scalar(yc0[:], yc0[:], 55.0, 56.0, OP.min, OP.mult)
                nc.vector.tensor_scalar(yc1[:], y0f[:], -1.0, 0.0, OP.add, OP.max)
                nc.vector.tensor_scalar(yc1[:], yc1[:], 55.0, 56.0, OP.min, OP.mult)
                nc.vector.tensor_scalar(xc0[:], x0f[:], -2.0, 0.0, OP.add, OP.max)
                nc.vector.tensor_scalar(xc0[:], xc0[:], 55.0, None, OP.min)
                nc.vector.tensor_scalar(xc1[:], x0f[:], -1.0, 0.0, OP.add, OP.max)
                nc.vector.tensor_scalar(xc1[:], xc1[:], 55.0, None, OP.min)

                If_g = qpool.tile([LC, 4 * NLC], F32R, tag="If_g", bufs=2,
                                  name="If_g")
                Ifv = If_g[:].rearrange("p (r c) -> p r c", r=4)
                nc.vector.tensor_tensor(Ifv[:, 0, :], yc0[:], xc0[:], op=OP.add)
                nc.vector.tensor_tensor(Ifv[:, 1, :], yc0[:], xc1[:], op=OP.add)
                nc.vector.tensor_tensor(Ifv[:, 2, :], yc1[:], xc0[:], op=OP.add)
                nc.vector.tensor_tensor(Ifv[:, 3, :], yc1[:], xc1[:], op=OP.add)

                # ---- idx wrap via PE selector matmuls (no DRAM roundtrip) ----
                pidx = pre1.tile([128, 196], F32, tag="pidx", bufs=1, name="pidx")
                pidxv = pidx[:].rearrange("p (ph rc) -> p ph rc", ph=7)
                for ph in range(7):
                    nc.tensor.matmul(
                        pidxv[:, ph, :],
                        sel_sb[:, 128 * ph:128 * (ph + 1)],
                        If_g[:],
                        start=True, stop=True,
                    )
                idxw = xpool.tile([128, 196], I16, tag="idxw", bufs=3, name="idxw")
                nc.vector.tensor_copy(
                    idxw[:].rearrange("p (rc ph) -> p ph rc", ph=7), pidxv[:])

                # ---- bilinear weight rows: DRAM flatten + PE broadcast ----
                nc.sync.dma_start(wgt_v[g], Wt_g[:].rearrange(
                    "p (r c) -> p r c", r=4))
                wrow1 = xpool.tile([1, 4 * L], F32R, tag="wrow1", bufs=1,
                                   name="wrow1")
                nc.sync.dma_start(wrow1[:], wrow_v[g])
                wbc = []
                for r in range(4):
                    wb = xpool.tile([128, L], F32, tag="wbc", bufs=6, name="wbc")
                    for n2 in range(2):
                        pwb = pre1.tile([128, 392], F32, tag="pwb", bufs=1,
                                        name="pwb")
                        nc.tensor.matmul(
                            pwb[:], ones128[:],
                            wrow1[0:1, L * r + 392 * n2:L * r + 392 * (n2 + 1)],
                            start=True, stop=True,
                        )
                        nc.scalar.activation(wb[:, 392 * n2:392 * (n2 + 1)],
                                             pwb[:], AF.Copy)
                    wbc.append(wb)

                # ---- gather + in-place bilinear combine ----
                gat = xpool.tile([128, 4 * L], F32, tag="gat", bufs=1, name="gat")
                nc.gpsimd.ap_gather(
                    gat[:], x_sb[g][:].bitcast(F32), idxw[:],
                    channels=128, num_elems=HW, d=1, num_idxs=4 * L,
                )
                gv = gat[:].rearrange("p (r n) -> p r n", r=4)
                for r in range(4):
                    nc.vector.tensor_tensor(gv[:, r, :], gv[:, r, :], wbc[r][:],
                                            op=OP.mult)
                nc.vector.tensor_tensor(gv[:, 0, :], gv[:, 0, :], gv[:, 1, :],
                                        op=OP.add)
                nc.vector.tensor_tensor(gv[:, 2, :], gv[:, 2, :], gv[:, 3, :],
                                        op=OP.add)
                nc.vector.tensor_tensor(xs_sb[g][:], gv[:, 0, :], gv[:, 2, :],
                                        op=OP.add)

            dbg_out("q", q_sb)
            dbg_out("xs", xs_sb)

        xctx.close()   # release x / gather staging space

        # ---------------- phase H: k and v^T (bf16) ----------------
        k_sb = [qpool.tile([128, L], BF16, name=f"k_sb{m}") for m in range(3)]
        vTe = [qpool.tile([LC, 6 * 65], BF16, name=f"vTe{lc}") for lc in range(NLC)]
        with tc.tile_pool(name="prepsum2", bufs=1, space="PSUM") as pre2:
            for m in range(3):
                for nn in range(2):
                    pk = pre2.tile([128, 392], F32, tag="pk", bufs=2, name="pk")
                    for kc in range(3):
                        nc.tensor.matmul(
                            pk[:],
                            _w3(kwk_b, kc, 128 * m, 128 * (m + 1)),
                            xs_sb[kc][:, 392 * nn:392 * (nn + 1)],
                            start=(kc == 0), stop=(kc == 2),
                        )
                    nc.vector.tensor_copy(k_sb[m][:, 392 * nn:392 * (nn + 1)], pk[:])

            for lc in range(NLC):
                nc.vector.memset(vTe[lc][:].bitcast(mybir.dt.uint16), 0x3F80)
                pv = pre2.tile([LC, DIM], F32, tag="pv", bufs=2, name="pv")
                for kc in range(3):
                    nc.tensor.matmul(
                        pv[:],
                        xs_sb[kc][:, LC * lc:LC * (lc + 1)],
                        _w3(kwv_b, kc, 0, DIM),
                        start=(kc == 0), stop=(kc == 2),
                    )
                dst = vTe[lc][:].rearrange("p (h d) -> p h d", h=6)[:, :, 0:64]
                nc.vector.tensor_copy(dst, pv[:].rearrange("p (h d) -> p h d", h=6))

        # memset 0x3F80 writes bf16 1.0 everywhere; cols 0:64 overwritten above

        dbg_out("k", k_sb)
        dbg_out("v", vTe)

        # ---------------- phase I: attention ----------------
        ot_all = qpool.tile([128, NOCH * DIM], BF16, name="ot_all")
        otv = ot_all[:].rearrange("p (c n) -> p c n", c=NOCH)
        OPp = oppsum.tile([128, 7 * 65], F32, name="OPp")
        OPv = OPp[:].rearrange("p (c e) -> p c e", c=7)

        def o_round(h, rnd, Eh):
            c0, c1 = ORND[rnd]
            nch = c1 - c0
            for ci in range(nch):
                c = c0 + ci
                qn = 128 if c < NOCH - 1 else 64
                for lc in range(NLC):
                    nc.tensor.matmul(
                        OPv[0:qn, ci, :],
                        Eh[lc][:, OCH * c:OCH * c + qn],
                        vTe[lc][:].rearrange("p (hh d) -> p hh d", hh=6)[:, h, :],
                        start=(lc == 0), stop=(lc == NLC - 1),
                    )
            # normalize: rec = 1/denom  (denom = col 64), out = O * rec
            nfull = nch if c1 < NOCH else nch - 1
            rec = qpool.tile([128, 7], F32, tag="rec", bufs=3, name="rec")
            if nfull > 0:
                nc.vector.reciprocal(rec[:, 0:nfull], OPv[:, 0:nfull, 64:65])
                nc.vector.tensor_tensor(
                    otv[:, c0:c0 + nfull, 64 * h:64 * (h + 1)],
                    OPv[:, 0:nfull, 0:64],
                    rec[:, 0:nfull].rearrange("p (c o) -> p c o", o=1)
                       .to_broadcast([128, nfull, 64]),
                    op=OP.mult)
            if c1 == NOCH:  # trailing 64-row chunk
                ci = nch - 1
                nc.vector.reciprocal(rec[0:64, ci:ci + 1], OPv[0:64, ci, 64:65])
                nc.vector.tensor_tensor(
                    otv[0:64, NOCH - 1, 64 * h:64 * (h + 1)],
                    OPv[0:64, ci, 0:64],
                    rec[0:64, ci:ci + 1].rearrange("p (c o) -> p c o", o=1)
                       .to_broadcast([64, 1, 64]),
                    op=OP.mult)

        Eprev = None
        ectx = __import__("contextlib").ExitStack()
        epool = ectx.enter_context(tc.tile_pool(name="epool", bufs=1))
        with tc.tile_pool(name="spsum", bufs=1, space="PSUM") as spsum:
            SA = spsum.tile([LC, 4 * 512], F32, name="SA")
            SB = spsum.tile([LC, 3 * 512], F32, name="SB")
            SAv = SA[:].rearrange("p (i n) -> p i n", i=4)
            SBv = SB[:].rearrange("p (i n) -> p i n", i=3)
            for h in range(NUM_HEAD):
                m2, hh = h // 2, h % 2
                kp = k_sb[m2][64 * hh:64 * hh + 64, :]
                qp = q_sb[m2][64 * hh:64 * hh + 64, :]
                Eh = []
                for lc in range(NLC):
                    for qi in range(NQC):
                        dst = SAv[:, qi, 0:QC] if qi < 4 else SBv[:, qi - 4, 0:QC]
                        nc.tensor.matmul(
                            dst,
                            kp[:, LC * lc:LC * (lc + 1)],
                            qp[:, QC * qi:QC * (qi + 1)],
                            start=True, stop=True,
                        )
                    E = epool.tile([LC, HW], BF16, tag=f"E{lc}", bufs=2, name="E")
                    Ev = E[:].rearrange("p (i n) -> p i n", i=NQC)
                    nc.scalar.activation(Ev[:, 0:4, :], SAv[:, :, 0:QC], AF.Exp)
                    nc.scalar.activation(Ev[:, 4:7, :], SBv[:, :, 0:QC], AF.Exp)
                    Eh.append(E)
                    if Eprev is not None and lc < 4:
                        o_round(h - 1, lc, Eprev)
                Eprev = Eh

        # ---------- tail: O(head 5) + transpose + proj, interleaved ----------
        O_sb = [qpool.tile([128, HW], BF16, name=f"O_sb{j}") for j in range(3)]
        with tc.tile_pool(name="tailpsum", bufs=1, space="PSUM") as tpsum:

            def transpose_chunk(c):
                pn = 128 if c < NOCH - 1 else 64
                for j in range(3):
                    tp = tpsum.tile([128, 128], BF16, tag="tp", bufs=3, name="tp")
                    nc.tensor.transpose(tp[0:128, 0:pn],
                                        otv[0:pn, c, 128 * j:128 * (j + 1)],
                                        ident_b[0:pn, 0:pn])
                    nc.vector.tensor_copy(O_sb[j][:, OCH * c:OCH * c + pn],
                                          tp[0:128, 0:pn])

            def proj_n(n):
                for m in range(3):
                    pp = tpsum.tile([128, QC], F32, tag="pp", bufs=3, name="pp")
                    for kc in range(3):
                        nc.tensor.matmul(
                            pp[:],
                            _w3(pjw_b, kc, 128 * m, 128 * (m + 1)),
                            O_sb[kc][:, QC * n:QC * (n + 1)],
                            start=(kc == 0), stop=(kc == 2),
                        )
                    y = qpool.tile([128, QC], F32, tag="y", bufs=4, name="y")
                    nc.scalar.activation(y[:], pp[:], AF.Identity,
                                         bias=pjb_sb[:, m:m + 1])
                    nc.sync.dma_start(
                        out_d[128 * m:128 * (m + 1), QC * n:QC * (n + 1)], y[:])

            for rnd in range(4):
                o_round(NUM_HEAD - 1, rnd, Eprev)
                for c in range(*ORND[rnd]):
                    transpose_chunk(c)
                if rnd >= 1:
                    for n in (2 * (rnd - 1), 2 * (rnd - 1) + 1):
                        proj_n(n)
            for n in (4, 5, 6):
                proj_n(n)
        ectx.close()

        dbg_out("ot", ot_all)
        dbg_out("O", O_sb)


def host_prep(inputs):
    """Shared (per-core-identical) weight prep. Returns dict of np arrays."""
    f = np.float32
    q_w = np.asarray(inputs["q_w"], f)
    kv_w = np.asarray(inputs["kv_w"], f)
    proj_w = np.asarray(inputs["proj_w"], f)
    proj_b = np.asarray(inputs["proj_b"], f)
    dw_w = np.asarray(inputs["dw_w"], f)
    dw_b = np.asarray(inputs["dw_b"], f)
    bn_w = np.asarray(inputs["bn_w"], f)
    bn_b = np.asarray(inputs["bn_b"], f)
    bn_mean = np.asarray(inputs["bn_mean"], f)
    bn_var = np.asarray(inputs["bn_var"], f)
    pw_w = np.asarray(inputs["pw_w"], f)

    bn_s = (bn_w / np.sqrt(bn_var + BN_EPS)).astype(f)
    bn_t = ((dw_b - bn_mean) * bn_s + bn_b).astype(f)

    p = np.arange(LC)
    c = np.arange(NLC)
    ytab_col = (4 * c[None, :] + p[:, None] // 28 + 0.5 + 2.0 / A).astype(f)
    ytab = np.tile(ytab_col, (1, G))                       # [112, 21] (g, c)
    xtab_col = (p % 28 + 0.5 + 2.0 / A).astype(f)[:, None]
    xtab = np.tile(xtab_col, (1, G * NLC))

    import ml_dtypes
    bf = ml_dtypes.bfloat16
    dwf = dw_w.reshape(NGD, 25)
    diag = np.zeros((NGD, 25, 128), f)
    diag[np.arange(128), :, np.arange(128)] = dwf
    # idx-wrap selector: sel[k, ph*128 + j] = 1 iff k == 16*ph + (j % 16)
    sel = np.zeros((LC, 7, 128), f)
    k_i = np.arange(LC)
    for ph in range(7):
        for j in range(128):
            sel[16 * ph + (j % 16), ph, j] = 1.0
    sel = np.ascontiguousarray(sel.reshape(LC, 7 * 128))
    return {
        "qw_t": np.ascontiguousarray(q_w.T).astype(bf),
        "kwk_t": np.ascontiguousarray((kv_w[:DIM] * SCALE).T).astype(bf),
        "kwv_t": np.ascontiguousarray(kv_w[DIM:].T).astype(bf),
        "pw_t": np.ascontiguousarray(pw_w.T),
        "projw_t": np.ascontiguousarray(proj_w.T).astype(bf),
        "projb_rs": np.ascontiguousarray(proj_b.reshape(3, NGD).T),
        "diag": np.ascontiguousarray(diag.reshape(NGD, 25 * 128)).astype(bf),
        "bn_s": bn_s.reshape(NGD, 1),
        "bn_t": bn_t.reshape(NGD, 1),
        "ident_b": np.eye(128, dtype=f).astype(bf),
        "ytA": (ytab * A).astype(f),
        "xtA": (xtab * A).astype(f),
        "sel": sel,
    }


_NC_CACHE = {}


def _get_nc(dummy=True):
    if "nc" not in _NC_CACHE:
        _NC_CACHE["nc"] = build_nc()
    return _NC_CACHE["nc"]


def make_in_maps(inputs):
    shared = host_prep(inputs)
    x = np.asarray(inputs["x"], np.float32)
    in_maps = []
    for i in range(B):
        m = dict(shared)
        xi = np.ascontiguousarray(x[i].reshape(DIM, HW))
        m["x"] = xi
        m["x_bf"] = xi.astype(ml_dtypes.bfloat16)
        in_maps.append(m)
    return in_maps


def run_spmd(inputs, trace=False):
    """Run on the 8 NeuronCores; returns (out (8,384,56,56), BassKernelResults)."""
    nc = _get_nc()
    in_maps = make_in_maps(inputs)
    res = bass_utils.run_bass_kernel_spmd(
        nc, in_maps, core_ids=list(range(B)), trace=trace,
    )
    out = np.stack([r["out"].reshape(DIM, H, W) for r in res.results], axis=0)
    return out, res


def kernel(**inputs) -> np.ndarray:
    out, _ = run_spmd(inputs, trace=False)
    return out
